# revision 4
# baseline (speedup 1.0000x reference)
"""Trainium2 Bass kernel: DepthSeparableConv2d block.

reference semantics:
    y = relu(bn1(depthwise3x3(x) + dw_b));  y = prune(y, 4.0)   per (b,c)
    z = relu(bn2(pointwise1x1(y) + pw_b));  z = prune(z, 0.001) per (b,o)

Strategy (8 NeuronCores, data-parallel over batch):
  - BN affines are folded into conv weights/biases on the host.
  - Depthwise conv: 9 per-partition-scalar MAC taps over a padded
    [128, 58*58] SBUF image (channel = partition).  tap0 is a 2x-mode
    tensor_scalar (mult+bias add); remaining taps are scalar_tensor_tensor
    in-place accumulations split between VectorE and GpSimd.
  - prune1: reduce_max over pre-relu accumulation (equivalent since
    thr > 0), mask folded into the pointwise lhsT (zero rows).
  - pointwise: PE matmul, lhsT=[C,O-half] fp32, rhs = relu(y) tiles.
  - BN2+relu fused into one ScalarE activation per PSUM tile with
    accum_out giving per-tile sums; prune2 uses sum>=thr which matches
    max>=thr except for channels whose every element is < thr (error
    bounded by thr = 1e-3, and never wrongly prunes).
"""

import os
import sys

import numpy as np

sys.path.insert(0, "/opt/trn_rl_repo")

import concourse.bacc as bacc  # noqa: E402
import concourse.tile as tile  # noqa: E402
from concourse import mybir  # noqa: E402
from concourse.bass_utils import run_bass_kernel_spmd  # noqa: E402


def _install_ntff_hook():
    """Register the axon NTFF profile hook (the image's antenv lacks
    axon_hooks, so trace=True would otherwise silently skip profiling)."""
    import types

    if "antenv.axon_hooks" in sys.modules:
        return
    mod = types.ModuleType("antenv.axon_hooks")
    state = {"hook": None}
    mod.set_axon_ntff_profile_hook = lambda h: state.__setitem__("hook", h)
    mod.get_axon_ntff_profile_hook = lambda: state["hook"]
    sys.modules["antenv.axon_hooks"] = mod
    try:
        if "/root/.axon_site" not in sys.path:
            sys.path.append("/root/.axon_site")
        from trn_agent_boot.trn_boot import _ntff_profile_via_ctypes

        hook = _ntff_profile_via_ctypes("/opt/axon/libaxon_pjrt.so")
        mod.set_axon_ntff_profile_hook(hook)
    except Exception:
        pass


_install_ntff_hook()

EPS = 1e-5
DW_THR = 4.0
PW_THR = 0.001

N_CORES = 8
B, C, O, H, W = 64, 128, 256, 56, 56
BL = B // N_CORES  # batches per core
HP = H + 2  # padded rows/cols (58)
S = H * W  # 3136
TSP = 448  # spatial tile (8 rows of 56)
NT = S // TSP  # 7

# engine for each of the 9 depthwise taps ('v' = VectorE; GpSimd lacks
# TensorScalarPtr support in walrus codegen)
TAP_ENGINES = ["v", "v", "v", "v", "v", "v", "v", "v", "v"]

_CACHE: dict = {}


def build_nc():
    f32 = mybir.dt.float32
    AX = mybir.AxisListType
    AL = mybir.AluOpType
    AF = mybir.ActivationFunctionType

    nc = bacc.Bacc(
        "TRN2",
        target_bir_lowering=False,
        debug=False,
        num_devices=N_CORES,
    )

    x_d = nc.dram_tensor("x", [BL, C, H, W], f32, kind="ExternalInput").ap()
    par_d = nc.dram_tensor("par", [C, 16], f32, kind="ExternalInput").ap()
    pw_d = nc.dram_tensor("pw", [C, O], f32, kind="ExternalInput").ap()
    z_d = nc.dram_tensor("z", [BL, O, H, W], f32, kind="ExternalOutput").ap()

    with tile.TileContext(nc) as tc:
        with (
            tc.tile_pool(name="const", bufs=1) as cpool,
            tc.tile_pool(name="xp", bufs=2) as xpool,
            tc.tile_pool(name="y", bufs=2) as ypool,
            tc.tile_pool(name="yr", bufs=2) as yrpool,
            tc.tile_pool(name="zh", bufs=3) as zpool,
            tc.tile_pool(name="wb", bufs=2) as wbpool,
            tc.tile_pool(name="sm", bufs=24) as smpool,
            tc.tile_pool(name="ps", bufs=6, space="PSUM") as pspool,
        ):
            par = cpool.tile([C, 16], f32, tag="par")
            nc.sync.dma_start(par[:], par_d)
            pw = cpool.tile([C, O], f32, tag="pw")
            nc.sync.dma_start(pw[:], pw_d)

            for b in range(BL):
                xp = xpool.tile([C, HP * HP], f32, tag="xp")
                xv = xp[:].rearrange("p (h w) -> p h w", h=HP)
                # zero the pad border: top row, bottom row, and the two
                # middle columns (w=57 and w=0 of the next row are adjacent
                # in the flat layout -> one strided memset covers both).
                nc.vector.memset(xv[:, 0:1, :], 0.0)
                nc.vector.memset(xv[:, HP - 1 : HP, :], 0.0)
                mid = xp[:, 57 : 57 + 57 * HP].rearrange(
                    "p (h w) -> p h w", w=HP
                )
                nc.vector.memset(mid[:, :, 0:2], 0.0)
                nc.sync.dma_start(xv[:, 1 : H + 1, 1 : W + 1], x_d[b])

                y = ypool.tile([C, S], f32, tag="y")
                yv = y[:].rearrange("p (h w) -> p h w", h=H)
                for k in range(9):
                    ky, kx = divmod(k, 3)
                    xin = xv[:, ky : ky + H, kx : kx + W]
                    if k == 0:
                        nc.vector.tensor_scalar(
                            yv, xin, par[:, 0:1], par[:, 9:10], AL.mult, AL.add
                        )
                    else:
                        eng = nc.vector if TAP_ENGINES[k] == "v" else nc.gpsimd
                        eng.scalar_tensor_tensor(
                            yv, xin, par[:, k : k + 1], yv, AL.mult, AL.add
                        )

                # prune1 mask from the pre-relu max (thr > 0 makes this
                # equivalent to the post-relu max test)
                m1 = smpool.tile([C, 1], f32, tag="m1")
                nc.vector.tensor_reduce(m1[:], y[:], AX.X, AL.max)
                k1 = smpool.tile([C, 1], f32, tag="k1")
                nc.vector.tensor_scalar(k1[:], m1[:], DW_THR, None, AL.is_ge)
                wb = wbpool.tile([C, O], f32, tag="wb")
                nc.vector.tensor_scalar(wb[:], pw[:], k1[:], None, AL.mult)

                yr = yrpool.tile([C, S], f32, tag="yr")
                nc.scalar.activation(yr[:], y[:], AF.Relu)

                for o2 in range(2):
                    zh = zpool.tile([C, S], f32, tag="zh")
                    zs = smpool.tile([C, NT], f32, tag="zs")
                    for j in range(NT):
                        ps = pspool.tile([C, TSP], f32, tag="ps")
                        nc.tensor.matmul(
                            ps[:],
                            lhsT=wb[:, o2 * C : (o2 + 1) * C],
                            rhs=yr[:, j * TSP : (j + 1) * TSP],
                            start=True,
                            stop=True,
                        )
                        nc.scalar.activation(
                            zh[:, j * TSP : (j + 1) * TSP],
                            ps[:],
                            AF.Relu,
                            bias=par[:, 10 + o2 : 11 + o2],
                            scale=1.0,
                            accum_out=zs[:, j : j + 1],
                        )
                    zt = smpool.tile([C, 1], f32, tag="zt")
                    nc.vector.tensor_reduce(zt[:], zs[:], AX.X, AL.add)
                    k2 = smpool.tile([C, 1], f32, tag="k2")
                    nc.vector.tensor_scalar(k2[:], zt[:], PW_THR, None, AL.is_ge)
                    nc.vector.tensor_scalar(zh[:], zh[:], k2[:], None, AL.mult)
                    nc.sync.dma_start(
                        z_d[b, o2 * C : (o2 + 1) * C],
                        zh[:].rearrange("p (h w) -> p h w", h=H),
                    )

    nc.compile()
    return nc


def fold_params(inp: dict) -> tuple[np.ndarray, np.ndarray]:
    """Fold BN affines into conv weights/biases (float64 folds)."""
    f8 = np.float64
    dw_w = np.asarray(inp["dw_w"], f8)  # [C,1,3,3]
    dw_b = np.asarray(inp["dw_b"], f8)
    g1, b1, m1, v1 = (np.asarray(inp[k], f8) for k in ("g1", "b1", "m1", "v1"))
    pw_w = np.asarray(inp["pw_w"], f8)  # [O,C,1,1]
    pw_b = np.asarray(inp["pw_b"], f8)
    g2, b2, m2, v2 = (np.asarray(inp[k], f8) for k in ("g2", "b2", "m2", "v2"))

    inv1 = g1 / np.sqrt(v1 + EPS)  # [C]
    wtap = dw_w[:, 0].reshape(C, 9) * inv1[:, None]  # [C,9]
    b1p = dw_b * inv1 + (b1 - m1 * inv1)  # [C]

    inv2 = g2 / np.sqrt(v2 + EPS)  # [O]
    lhsT = (pw_w[:, :, 0, 0] * inv2[:, None]).T  # [C,O]
    b2p = pw_b * inv2 + (b2 - m2 * inv2)  # [O]

    par = np.zeros((C, 16), np.float32)
    par[:, 0:9] = wtap.astype(np.float32)
    par[:, 9] = b1p.astype(np.float32)
    par[:, 10] = b2p[:C].astype(np.float32)
    par[:, 11] = b2p[C:].astype(np.float32)
    return par, lhsT.astype(np.float32)


def kernel(**inputs) -> np.ndarray:
    x = np.ascontiguousarray(np.asarray(inputs["x"], np.float32))
    assert x.shape == (B, C, H, W)
    par, pw = fold_params(inputs)

    if "nc" not in _CACHE:
        _CACHE["nc"] = build_nc()
    nc = _CACHE["nc"]

    in_maps = [
        {"x": x[i * BL : (i + 1) * BL], "par": par, "pw": pw}
        for i in range(N_CORES)
    ]
    trace = bool(int(os.environ.get("KERNEL_TRACE", "0")))
    res = run_bass_kernel_spmd(nc, in_maps, list(range(N_CORES)), trace=trace)
    _CACHE["last_exec_time_ns"] = res.exec_time_ns

    z = np.empty((B, O, H, W), np.float32)
    for i in range(N_CORES):
        z[i * BL : (i + 1) * BL] = res.results[i]["z"]
    return z


# revision 5
# speedup vs baseline: 1.3039x; 1.3039x over previous
"""Trainium2 Bass kernel: DepthSeparableConv2d block.

reference semantics:
    y = relu(bn1(depthwise3x3(x) + dw_b));  y = prune(y, 4.0)   per (b,c)
    z = relu(bn2(pointwise1x1(y) + pw_b));  z = prune(z, 0.001) per (b,o)

Strategy (8 NeuronCores, data-parallel over batch; channel = partition):
  - BN affines folded into conv weights/biases on the host (float64).
  - Depthwise 3x3 over a padded [128, 58*58] SBUF image:
      * tap0 on VectorE tensor_scalar (2x mode, carries the bias),
      * taps 1..5 on VectorE scalar_tensor_tensor in-place MACs (fp32),
      * taps 6..8 on TensorE as fp32 diag-weight matmuls accumulating in
        PSUM per 448-wide spatial tile,
      * a custom DVE op merges PSUM + SBUF accumulators, applies ReLU,
        and max-reduces per partition in ONE 1x pass (prune1 for free).
  - prune1 mask folded into the pointwise lhsT (zeroed rows).
  - pointwise matmul in float32r (1 cyc/row vs fp32's 4; HW-measured
    rel err ~1.5e-4 of K=128 dot scale, well inside tolerance here).
  - BN2+relu fused into one ScalarE activation per PSUM tile with
    accum_out per-tile sums; prune2 via sum>=thr (== max>=thr except for
    all-tiny channels; error bounded by thr=1e-3 and never over-prunes).
  - prune2 mask applied by ScalarE (activation Copy with per-partition
    scale) to keep VectorE free.
"""

import os
import sys

import numpy as np

sys.path.insert(0, "/opt/trn_rl_repo")

import concourse.bacc as bacc  # noqa: E402
import concourse.tile as tile  # noqa: E402
from concourse import mybir  # noqa: E402
from concourse.bass_utils import run_bass_kernel_spmd  # noqa: E402


def _install_ntff_hook():
    """Register the axon NTFF profile hook (the image's antenv lacks
    axon_hooks, so trace=True would otherwise silently skip profiling)."""
    import types

    if "antenv.axon_hooks" in sys.modules:
        return
    mod = types.ModuleType("antenv.axon_hooks")
    state = {"hook": None}
    mod.set_axon_ntff_profile_hook = lambda h: state.__setitem__("hook", h)
    mod.get_axon_ntff_profile_hook = lambda: state["hook"]
    sys.modules["antenv.axon_hooks"] = mod
    try:
        if "/root/.axon_site" not in sys.path:
            sys.path.append("/root/.axon_site")
        from trn_agent_boot.trn_boot import _ntff_profile_via_ctypes

        hook = _ntff_profile_via_ctypes("/opt/axon/libaxon_pjrt.so")
        mod.set_axon_ntff_profile_hook(hook)
    except Exception:
        pass


_install_ntff_hook()

EPS = 1e-5
DW_THR = 4.0
PW_THR = 0.001

N_CORES = 8
B, C, O, H, W = 64, 128, 256, 56, 56
BL = B // N_CORES  # batches per core
HP = H + 2  # padded rows/cols (58)
S = H * W  # 3136
TSP = 448  # spatial tile (8 rows of 56)
NT = S // TSP  # 7

PE_TAPS = (6, 7, 8)  # depthwise taps computed on TensorE (fp32 diag matmul)
DVE_STT_TAPS = (1, 2, 3, 4, 5)

_CACHE: dict = {}


def _register_fused_op():
    """Custom DVE op: out = relu(in0*s0 + in1); accum_out = max(s1, max(out)).

    Used as the depthwise merge: in0 = PSUM partial (PE taps), s0 = 1.0,
    in1 = SBUF partial (DVE taps).  One 1x VectorE pass replaces
    {PSUM merge, ScalarE relu pass, VectorE reduce_max} and feeds prune1.
    """
    from concourse import dve_ops as dvo
    from concourse.dve_spec import C0, C1, Spec, Src0, Src1, lower, maxx, relu
    from concourse.dve_uop import DveOpSpec

    name = "AFFINE_ADD_RELU_MAXACC_ANT"
    if name in dvo._SUB_OPCODE_FOR_NAME:
        return next(op for op in dvo.OPS if op.name == name)

    def ref(in0, in1, s0, s1, imm2):
        out = np.maximum(in0.astype(np.float32) * s0 + in1, 0.0)
        acc = np.maximum(
            out.reshape(out.shape[0], -1).max(axis=-1, keepdims=True), s1
        )
        return out, acc

    spec = Spec(body=relu(Src0 * C0 + Src1), accum=maxx, accum_init=C1, reference=ref)
    row = dvo._CUSTOM_DVE_ROW_BASE + len(dvo.OPS)
    shas = {
        ver: DveOpSpec(
            name=name, opcode=row, uops=lower(spec, ver=ver), rd1_en=True
        ).sha(ver)
        for ver in ("v3", "v4")
    }
    op = dvo.DveOp(name, spec, subdim=False, uops_sha=shas)
    dvo.OPS.append(op)
    dvo.CUSTOM_DVE_SPECS[name] = spec
    dvo._SUB_OPCODE_FOR_NAME[name] = row
    return op


def build_nc():
    f32 = mybir.dt.float32
    f32r = mybir.dt.float32r
    AX = mybir.AxisListType
    AL = mybir.AluOpType
    AF = mybir.ActivationFunctionType
    fused_op = _register_fused_op()

    nc = bacc.Bacc(
        "TRN2",
        target_bir_lowering=False,
        debug=False,
        num_devices=N_CORES,
    )

    x_d = nc.dram_tensor("x", [BL, C, H, W], f32, kind="ExternalInput").ap()
    par_d = nc.dram_tensor("par", [C, 16], f32, kind="ExternalInput").ap()
    pw_d = nc.dram_tensor("pw", [C, O], f32, kind="ExternalInput").ap()
    dg_d = nc.dram_tensor(
        "dg", [C, len(PE_TAPS) * C], f32, kind="ExternalInput"
    ).ap()
    z_d = nc.dram_tensor("z", [BL, O, H, W], f32, kind="ExternalOutput").ap()

    with tile.TileContext(nc) as tc:
        with (
            tc.tile_pool(name="const", bufs=1) as cpool,
            tc.tile_pool(name="xp", bufs=2) as xpool,
            tc.tile_pool(name="y", bufs=2) as ypool,
            tc.tile_pool(name="yr", bufs=2) as yrpool,
            tc.tile_pool(name="zh", bufs=3) as zpool,
            tc.tile_pool(name="wb", bufs=2) as wbpool,
            tc.tile_pool(name="sm", bufs=32) as smpool,
            tc.tile_pool(name="pdw", bufs=3, space="PSUM") as pdwpool,
            tc.tile_pool(name="ppw", bufs=4, space="PSUM") as ppwpool,
        ):
            par = cpool.tile([C, 16], f32, tag="par")
            nc.sync.dma_start(par[:], par_d)
            pw = cpool.tile([C, O], f32, tag="pw")
            nc.sync.dma_start(pw[:], pw_d)
            dg = cpool.tile([C, len(PE_TAPS) * C], f32, tag="dg")
            nc.sync.dma_start(dg[:], dg_d)

            for b in range(BL):
                xp = xpool.tile([C, HP * HP], f32, tag="xp")
                xv = xp[:].rearrange("p (h w) -> p h w", h=HP)
                # zero the pad border: top row, bottom row, and the two
                # middle columns (w=57 and w=0 of the next row are adjacent
                # in the flat layout -> one strided memset covers both).
                nc.gpsimd.memset(xv[:, 0:1, :], 0.0)
                nc.gpsimd.memset(xv[:, HP - 1 : HP, :], 0.0)
                mid = xp[:, 57 : 57 + 57 * HP].rearrange(
                    "p (h w) -> p h w", w=HP
                )
                nc.gpsimd.memset(mid[:, :, 0:2], 0.0)
                nc.sync.dma_start(xv[:, 1 : H + 1, 1 : W + 1], x_d[b])

                # depthwise: VectorE accumulator (taps 0..5)
                y = ypool.tile([C, S], f32, tag="y")
                yv = y[:].rearrange("p (h w) -> p h w", h=H)
                ky, kx = divmod(0, 3)
                nc.vector.tensor_scalar(
                    yv,
                    xv[:, ky : ky + H, kx : kx + W],
                    par[:, 0:1],
                    par[:, 9:10],
                    AL.mult,
                    AL.add,
                )
                for k in DVE_STT_TAPS:
                    ky, kx = divmod(k, 3)
                    nc.vector.scalar_tensor_tensor(
                        yv,
                        xv[:, ky : ky + H, kx : kx + W],
                        par[:, k : k + 1],
                        yv,
                        AL.mult,
                        AL.add,
                    )

                # depthwise: TensorE taps into PSUM per spatial tile, then
                # the fused DVE op merges + relu + per-tile max (prune1).
                yr = yrpool.tile([C, S], f32r, tag="yr")
                m1s = smpool.tile([C, NT], f32, tag="m1s")
                for j in range(NT):
                    pdw = pdwpool.tile([C, TSP], f32, tag="pdw")
                    for t, k in enumerate(PE_TAPS):
                        ky, kx = divmod(k, 3)
                        nc.tensor.matmul(
                            pdw[:],
                            lhsT=dg[:, t * C : (t + 1) * C],
                            rhs=xv[:, 8 * j + ky : 8 * j + ky + 8, kx : kx + W],
                            start=(t == 0),
                            stop=(t == len(PE_TAPS) - 1),
                        )
                    nc.vector._custom_dve(
                        fused_op,
                        out=yr[:, j * TSP : (j + 1) * TSP],
                        in0=pdw[:],
                        in1=y[:, j * TSP : (j + 1) * TSP],
                        s0=1.0,
                        s1=0.0,
                        accum_out=m1s[:, j : j + 1],
                    )

                # prune1 mask -> masked pointwise weights (float32r)
                m1 = smpool.tile([C, 1], f32, tag="m1")
                nc.vector.tensor_reduce(m1[:], m1s[:], AX.X, AL.max)
                k1 = smpool.tile([C, 1], f32, tag="k1")
                nc.vector.tensor_scalar(k1[:], m1[:], DW_THR, None, AL.is_ge)
                wb = wbpool.tile([C, O], f32r, tag="wb")
                nc.vector.tensor_scalar(wb[:], pw[:], k1[:], None, AL.mult)

                for o2 in range(2):
                    zh = zpool.tile([C, S], f32, tag="zh")
                    zs = smpool.tile([C, NT], f32, tag="zs")
                    for j in range(NT):
                        ppw = ppwpool.tile([C, TSP], f32, tag="ppw")
                        nc.tensor.matmul(
                            ppw[:],
                            lhsT=wb[:, o2 * C : (o2 + 1) * C],
                            rhs=yr[:, j * TSP : (j + 1) * TSP],
                            start=True,
                            stop=True,
                        )
                        nc.scalar.activation(
                            zh[:, j * TSP : (j + 1) * TSP],
                            ppw[:],
                            AF.Relu,
                            bias=par[:, 10 + o2 : 11 + o2],
                            scale=1.0,
                            accum_out=zs[:, j : j + 1],
                        )
                    zt = smpool.tile([C, 1], f32, tag="zt")
                    nc.vector.tensor_reduce(zt[:], zs[:], AX.X, AL.add)
                    k2 = smpool.tile([C, 1], f32, tag="k2")
                    nc.vector.tensor_scalar(k2[:], zt[:], PW_THR, None, AL.is_ge)
                    # prune2 applied on ScalarE (Copy w/ per-partition scale)
                    nc.scalar.mul(zh[:], zh[:], k2[:])
                    nc.sync.dma_start(
                        z_d[b, o2 * C : (o2 + 1) * C],
                        zh[:].rearrange("p (h w) -> p h w", h=H),
                    )

    nc.compile()
    return nc


def fold_params(inp: dict):
    """Fold BN affines into conv weights/biases (float64 folds)."""
    f8 = np.float64
    dw_w = np.asarray(inp["dw_w"], f8)  # [C,1,3,3]
    dw_b = np.asarray(inp["dw_b"], f8)
    g1, b1, m1, v1 = (np.asarray(inp[k], f8) for k in ("g1", "b1", "m1", "v1"))
    pw_w = np.asarray(inp["pw_w"], f8)  # [O,C,1,1]
    pw_b = np.asarray(inp["pw_b"], f8)
    g2, b2, m2, v2 = (np.asarray(inp[k], f8) for k in ("g2", "b2", "m2", "v2"))

    inv1 = g1 / np.sqrt(v1 + EPS)  # [C]
    wtap = dw_w[:, 0].reshape(C, 9) * inv1[:, None]  # [C,9]
    b1p = dw_b * inv1 + (b1 - m1 * inv1)  # [C]

    inv2 = g2 / np.sqrt(v2 + EPS)  # [O]
    lhsT = (pw_w[:, :, 0, 0] * inv2[:, None]).T  # [C,O]
    b2p = pw_b * inv2 + (b2 - m2 * inv2)  # [O]

    par = np.zeros((C, 16), np.float32)
    par[:, 0:9] = wtap.astype(np.float32)
    par[:, 9] = b1p.astype(np.float32)
    par[:, 10] = b2p[:C].astype(np.float32)
    par[:, 11] = b2p[C:].astype(np.float32)

    dgm = np.zeros((C, len(PE_TAPS) * C), np.float32)
    for t, k in enumerate(PE_TAPS):
        dgm[np.arange(C), t * C + np.arange(C)] = wtap[:, k].astype(np.float32)
    return par, lhsT.astype(np.float32), dgm


def kernel(**inputs) -> np.ndarray:
    x = np.ascontiguousarray(np.asarray(inputs["x"], np.float32))
    assert x.shape == (B, C, H, W)
    par, pw, dgm = fold_params(inputs)

    if "nc" not in _CACHE:
        _CACHE["nc"] = build_nc()
    nc = _CACHE["nc"]

    in_maps = [
        {"x": x[i * BL : (i + 1) * BL], "par": par, "pw": pw, "dg": dgm}
        for i in range(N_CORES)
    ]
    trace = bool(int(os.environ.get("KERNEL_TRACE", "0")))
    res = run_bass_kernel_spmd(nc, in_maps, list(range(N_CORES)), trace=trace)
    _CACHE["last_exec_time_ns"] = res.exec_time_ns

    z = np.empty((B, O, H, W), np.float32)
    for i in range(N_CORES):
        z[i * BL : (i + 1) * BL] = res.results[i]["z"]
    return z
